# revision 42
# baseline (speedup 1.0000x reference)
"""Trainium2 Bass kernel for a ViT attention block (LN->MHA+relpos->LN->MLP).

Contract: kernel(**inputs) takes the FULL unsharded inputs, shards batch
across 8 NeuronCores (4 items per core), runs one SPMD Bass program, and
gathers the full [32, 577, 768] fp32 output.

v2 design (channel-major):
- All activations flow as [channel(partition), token(free)] slabs; the host
  ships x pre-transposed, so no DMA-xbar transposes on device.
- LayerNorm means are folded into the next matmul's weights as a rank-1
  correction (W' = W - rowmean(W)); LN1's rstd is computed on the host
  (it depends only on the input x), LN2's rstd via ones-matmul token sums.
- fp8(e4m3) DoubleRow matmuls for qkv, proj, PV, and fc1 (2x PE columns);
  fc2 stays bf16 for accuracy; S stays bf16.
- S matmuls are K=64 row-tiled: head pairs land on PE row-strips (0,0) and
  (64,0) and run concurrently.
- Relative-position bias is added into the S PSUM tile by the Pool engine
  (nc.gpsimd), not by identity matmuls; exp runs on Act straight out of
  PSUM into fp8.
- P^T @ [v|1] (fp8 DoubleRow over 6 m-chunk pairs) yields O^T plus the
  softmax denominator via an ones-column in the v slab.
- fc1/fc2 weights are streamed from DRAM per chunk (SBUF pressure).
"""

import sys

if '/opt/trn_rl_repo' not in sys.path:
    sys.path.insert(0, '/opt/trn_rl_repo')

from contextlib import ExitStack

import numpy as np
import ml_dtypes

import concourse.bass as bass  # noqa: F401
import concourse.tile as tile
import concourse.mybir as mybir
from concourse import bacc, bass_utils

BF16 = ml_dtypes.bfloat16
F8 = ml_dtypes.float8_e4m3   # TRN fp8e4 (e4m3, max 240)
F32 = np.float32

B = 32
N = 577
C = 768
NH = 12
HD = 64
MLP = 3072
EPS = 1e-6
SCALE = HD ** (-0.5)

N_CORES = 8
BPC = B // N_CORES          # 4 batch items per core
NPAD = 640                  # per-item padded token count (5 * 128)
KC = C // 128               # 6 contraction chunks for dim 768
MC = MLP // 128             # 24 chunks for MLP dim
MCHUNK = 5                  # m-chunks covering 577 tokens (4*128 + 65)
PW = 592                    # padded 577 (stride % 16 == 0)
VROW = 864                  # v slab row: 12 heads * 66 + pad

F32T = mybir.dt.float32
BF16T = mybir.dt.bfloat16
F8T = mybir.dt.float8e4
AF = mybir.ActivationFunctionType
OP = mybir.AluOpType
DR = mybir.MatmulPerfMode.DoubleRow

SPL_N = [(0, 512), (512, 65)]    # 577-wide streams
SPL_P = [(0, 512), (512, 128)]   # 640-wide streams

GELU_AF = AF.Gelu  # sim_test swaps this (CoreSim lacks Gelu); HW uses Gelu
ZERO_ALL_SLABS = False  # sim-only: defeat the pool-slot zero-persistence


def build_program(nc):
    dt = mybir.dt

    xT_d = nc.dram_tensor("xT8", [BPC, C, NPAD], dt.float8e4, kind="ExternalInput")
    xbT_d = nc.dram_tensor("xbT", [BPC, C, NPAD], dt.float32, kind="ExternalInput")
    rstd1_d = nc.dram_tensor("rstd1", [BPC, NPAD], dt.float32, kind="ExternalInput")
    wqk_d = nc.dram_tensor("wqk8", [C, 2 * C], dt.float8e4, kind="ExternalInput")
    wv_d = nc.dram_tensor("wv8", [C, C], dt.float8e4, kind="ExternalInput")
    wp_d = nc.dram_tensor("wp8", [C, C], dt.float8e4, kind="ExternalInput")
    w1_d = nc.dram_tensor("w18", [C, MLP], dt.float8e4, kind="ExternalInput")
    w2_d = nc.dram_tensor("w2T", [MLP, C], dt.bfloat16, kind="ExternalInput")
    b1_d = nc.dram_tensor("bias_fc1", [MLP], dt.float32, kind="ExternalInput")
    b2_d = nc.dram_tensor("bias_fc2", [C], dt.float32, kind="ExternalInput")
    rpb_d = nc.dram_tensor("rpb8", [KC, 128, 2, MCHUNK, PW], dt.float8e4,
                           kind="ExternalInput")
    out_d = nc.dram_tensor("outT", [BPC, C, NPAD], dt.float32,
                           kind="ExternalOutput")

    x2_d = nc.dram_tensor("x2T_scratch", [BPC, C, NPAD], dt.float32)
    rec_d = nc.dram_tensor("rec_scratch", [BPC, NH, PW], dt.float32)
    rsd2_d = nc.dram_tensor("rstd2_scratch", [BPC, NPAD], dt.float32)

    with tile.TileContext(nc) as tc, ExitStack() as ctx:
        psA = ctx.enter_context(tc.tile_pool(name="psA", bufs=2, space="PSUM"))
        psPV = ctx.enter_context(tc.tile_pool(name="psPV", bufs=1, space="PSUM"))
        psG = ctx.enter_context(tc.tile_pool(name="psG", bufs=1, space="PSUM"))

        persist = ctx.enter_context(tc.tile_pool(name="persist", bufs=1))
        wqk_sb = persist.tile([128, KC, 2 * C], F8T, tag="wqk")
        nc.sync.dma_start(wqk_sb[:], wqk_d.ap().rearrange("(k p) c -> p k c", p=128))
        wv_sb = persist.tile([128, KC, C], F8T, tag="wv")
        nc.sync.dma_start(wv_sb[:], wv_d.ap().rearrange("(k p) c -> p k c", p=128))
        wp_sb = persist.tile([128, KC, C], F8T, tag="wp")
        nc.sync.dma_start(wp_sb[:], wp_d.ap().rearrange("(k p) c -> p k c", p=128))
        ones_sb = persist.tile([128, 1], BF16T, tag="ones")
        nc.vector.memset(ones_sb[:], 1.0)
        eps_sb = persist.tile([128, 1], F32T, tag="eps")
        nc.vector.memset(eps_sb[:], EPS)
        bfc1_sb = persist.tile([128, MC], F32T, tag="bfc1")
        nc.sync.dma_start(bfc1_sb[:], b1_d.ap().rearrange("(m p) -> p m", p=128))
        bfc2_sb = persist.tile([128, KC], F32T, tag="bfc2")
        nc.sync.dma_start(bfc2_sb[:], b2_d.ap().rearrange("(m p) -> p m", p=128))

        xtp = ctx.enter_context(tc.tile_pool(name="xtp", bufs=2))
        qkp = ctx.enter_context(tc.tile_pool(name="qkp", bufs=2))
        kzp = ctx.enter_context(tc.tile_pool(name="kzp", bufs=2))
        v8p = ctx.enter_context(tc.tile_pool(name="v8p", bufs=2))
        ptp = ctx.enter_context(tc.tile_pool(name="ptp", bufs=2))
        rpbp = ctx.enter_context(tc.tile_pool(name="rpbp", bufs=2))
        osbp = ctx.enter_context(tc.tile_pool(name="osbp", bufs=2))
        onp = ctx.enter_context(tc.tile_pool(name="onp", bufs=2))
        xh2p = ctx.enter_context(tc.tile_pool(name="xh2p", bufs=2))
        mtp = ctx.enter_context(tc.tile_pool(name="mtp", bufs=1))
        w1tp = ctx.enter_context(tc.tile_pool(name="w1tp", bufs=2))
        w2tp = ctx.enter_context(tc.tile_pool(name="w2tp", bufs=2))
        ckp = ctx.enter_context(tc.tile_pool(name="ckp", bufs=2))
        ckm = ctx.enter_context(tc.tile_pool(name="ckm", bufs=2))
        rowp = ctx.enter_context(tc.tile_pool(name="rowp", bufs=2))
        rbp = ctx.enter_context(tc.tile_pool(name="rbp", bufs=2))
        smallp = ctx.enter_context(tc.tile_pool(name="smallp", bufs=2))
        denp = ctx.enter_context(tc.tile_pool(name="denp", bufs=1))

        # pool-slot zero-persistence counters (one per pool; slots alternate
        # and the zeroed regions are never overwritten by data)
        v8_cnt = [0]
        pt_cnt = [0]
        on_cnt = [0]

        # ---------------- qkv production for one item ----------------
        def qkv_steps(b):
            cell = {}

            def alloc_step():
                xt = xtp.tile([128, KC, NPAD], F8T, tag="xT")
                nc.sync.dma_start(
                    xt[:], xT_d.ap()[b].rearrange("(k p) t -> p k t", p=128))
                rrow = rowp.tile([128, NPAD], F32T, tag="rrow")
                src = rstd1_d.ap()[b]
                nc.sync.dma_start(rrow[:], bass.AP(
                    tensor=src.tensor, offset=src.offset,
                    ap=[[0, 128]] + list(src.ap)))
                rcol = smallp.tile([128, MCHUNK], F32T, tag="rcol")
                nc.sync.dma_start(
                    rcol[:], rstd1_d.ap()[b].rearrange("(c p) -> p c", p=128))
                qkT = qkp.tile([128, KC, PW], BF16T, tag="qkT")
                kz = kzp.tile([128, KC, PW], BF16T, tag="kz")
                v8 = v8p.tile([128, MCHUNK, VROW], F8T, tag="v8")
                if v8_cnt[0] < 2 or ZERO_ALL_SLABS:
                    v8_cnt[0] += 1
                    nc.vector.memset(v8[:], 0.0)
                    # ones columns (softmax denominator)
                    nc.vector.memset(
                        v8[:, :, 0:NH * 66].rearrange(
                            "p m (h e) -> p m h e", e=66)[:, :, :, 64:66], 1.0)
                cell.update(xT=xt, rrow=rrow, rcol=rcol, qkT=qkT, kz=kz, v8=v8)

            def qk_step(hp, which):
                ps = psG.tile([128, 1024], F32T, tag="ps")
                c0 = which * C + hp * 128
                for p in range(3):
                    for (lo, w) in SPL_N:
                        nc.tensor.matmul(
                            ps[:, lo:lo + w],
                            lhsT=wqk_sb[:, 2 * p:2 * p + 2, c0:c0 + 128],
                            rhs=cell['xT'][:, 2 * p:2 * p + 2, lo:lo + w],
                            start=(p == 0), stop=(p == 2), perf_mode=DR)
                dest = cell['qkT'] if which == 0 else cell['kz']
                nc.vector.tensor_tensor(
                    dest[:, hp, 0:N], ps[:, 0:N], cell['rrow'][:, 0:N], OP.mult)

            def v_step(mc):
                mw = 128 if mc < MCHUNK - 1 else N - 512
                ps = psG.tile([128, 1024], F32T, tag="ps")
                for p in range(3):
                    for (lo, w) in [(0, 512), (512, 256)]:
                        nc.tensor.matmul(
                            ps[0:mw, lo:lo + w],
                            lhsT=cell['xT'][:, 2 * p:2 * p + 2,
                                            mc * 128:mc * 128 + mw],
                            rhs=wv_sb[:, 2 * p:2 * p + 2, lo:lo + w],
                            start=(p == 0), stop=(p == 2), perf_mode=DR)
                nc.vector.tensor_scalar(
                    out=cell['v8'][0:mw, mc, 0:NH * 66].rearrange(
                        "p (h e) -> p h e", e=66)[:, :, 0:64],
                    in0=ps[0:mw, 0:C].rearrange("p (h e) -> p h e", h=NH),
                    scalar1=cell['rcol'][0:mw, mc:mc + 1], scalar2=None,
                    op0=OP.mult)

            steps = [alloc_step]
            for hp in range(KC):
                steps.append(lambda hp=hp: qk_step(hp, 0))
                steps.append(lambda hp=hp: qk_step(hp, 1))
            for mc in range(MCHUNK):
                steps.append(lambda mc=mc: v_step(mc))
            return steps, cell

        # ---------------- PV drain context ----------------
        class PvCtx:
            def __init__(self, pt, v8, h, osb, den12):
                self.pt, self.v8, self.h = pt, v8, h
                self.osb, self.den12 = osb, den12
                self.pv = None
                self.mms = [(p, lo, w) for p in range(3) for (lo, w) in SPL_N]
                self.pos = 0

            def drain(self, k):
                if self.pv is None:
                    self.pv = psPV.tile([128, 1024], F32T, tag="pv")
                h, j = self.h, self.h % 2
                w0 = h * 66
                mtail = N - 512
                end = min(self.pos + k, len(self.mms))
                for (p, lo, w) in self.mms[self.pos:end]:
                    if p < 2:
                        nc.tensor.matmul(
                            self.pv[:, lo:lo + w],
                            lhsT=self.v8[:, 2 * p:2 * p + 2, w0:w0 + 128],
                            rhs=self.pt[:, j, 2 * p:2 * p + 2, lo:lo + w],
                            start=(p == 0), stop=False, perf_mode=DR)
                    else:
                        nc.tensor.matmul(
                            self.pv[:, lo:lo + w],
                            lhsT=self.v8[0:mtail, 4, w0:w0 + 128],
                            rhs=self.pt[0:mtail, j, 4, lo:lo + w],
                            start=False, stop=True)
                self.pos = end
                if self.pos == len(self.mms):
                    hp = h // 2
                    dd = smallp.tile([1, PW], F32T, tag="dd")
                    nc.scalar.activation(dd[:, 0:N], self.pv[64:65, 0:N],
                                         AF.Identity)
                    nc.sync.dma_start(
                        self.den12[h:h + 1, 0:N], dd[:, 0:N])
                    nc.scalar.activation(
                        self.osb[64 * j:64 * j + 64, hp, 0:N],
                        self.pv[0:64, 0:N], AF.Identity)
                    self.pv = None
                    return True
                return False

            def finish(self):
                while self.pos < len(self.mms):
                    self.drain(6)

        # ---------------- attention phase for one item ----------------
        def attn_phase(b, cell, sec_mid, sec_tail):
            qkT, kz, v8 = cell['qkT'], cell['kz'], cell['v8']
            osb = osbp.tile([128, KC, PW], F8T, tag="osb")
            den12 = denp.tile([NH, PW], F32T, tag="den12")
            rts = {}

            def load_rpb(hp):
                rt = rpbp.tile([128, 2, MCHUNK, PW], F8T, tag="rpb")
                nc.sync.dma_start(rt[:], rpb_d.ap()[hp])
                rts[hp] = rt

            load_rpb(0)
            pend = []
            sec_i = [0]

            def run_sec(n):
                for _ in range(n):
                    if sec_i[0] < len(sec_mid):
                        sec_mid[sec_i[0]]()
                        sec_i[0] += 1

            for hp in range(KC):
                if hp + 1 < KC:
                    load_rpb(hp + 1)
                pt = ptp.tile([128, 2, MCHUNK, PW], F8T, tag="pt")
                for mc in range(MCHUNK):
                    mw = 128 if mc < MCHUNK - 1 else N - 512
                    m0 = mc * 128
                    for j in (0, 1):
                        sps = psA.tile([128, 1024], F32T, tag="ps")
                        b0 = 64 * j
                        for (lo, w) in SPL_N:
                            nc.tensor.matmul(
                                sps[0:mw, lo:lo + w],
                                lhsT=kz[b0:b0 + 64, hp, m0:m0 + mw],
                                rhs=qkT[b0:b0 + 64, hp, lo:lo + w],
                                start=True, stop=True)
                        nc.vector.tensor_tensor(
                            sps[0:mw, 0:N], sps[0:mw, 0:N],
                            rts[hp][0:mw, j, mc, 0:N], OP.add)
                        nc.scalar.activation(
                            pt[0:mw, j, mc, 0:N], sps[0:mw, 0:N], AF.Exp)
                    if pend:
                        if pend[0].drain(2):
                            pend.pop(0)
                    run_sec(1)
                pend.append(PvCtx(pt, v8, 2 * hp, osb, den12))
                pend.append(PvCtx(pt, v8, 2 * hp + 1, osb, den12))
                while len(pend) > 2:
                    if pend[0].drain(6):
                        pend.pop(0)
            while pend:
                if pend[0].drain(6):
                    pend.pop(0)
                run_sec(1)
            rec12 = denp.tile([NH, PW], F32T, tag="rec12")
            nc.vector.reciprocal(rec12[:, 0:N], den12[:, 0:N])
            nc.sync.dma_start(rec_d.ap()[b][:, 0:N], rec12[:, 0:N])
            while sec_i[0] < len(sec_mid):
                sec_mid[sec_i[0]]()
                sec_i[0] += 1
            for s in sec_tail:
                s()
            return dict(osb=osb, b=b)

        # ---------------- finalize (runs inside next attn phase) -------
        def finalize_steps(fc):
            b, osb = fc['b'], fc['osb']
            onorm = onp.tile([128, KC, PW], F8T, tag="onorm")
            mid = []

            def mult_pair(hp):
                rb = rbp.tile([128, 2, PW], F32T, tag="rb")
                src = rec_d.ap()[b][2 * hp:2 * hp + 2, 0:N]
                nc.sync.dma_start(rb[:, :, 0:N], bass.AP(
                    tensor=src.tensor, offset=src.offset,
                    ap=[[0, 128]] + list(src.ap)))
                for j in (0, 1):
                    nc.vector.tensor_tensor(
                        onorm[64 * j:64 * j + 64, hp, 0:N],
                        osb[64 * j:64 * j + 64, hp, 0:N],
                        rb[64 * j:64 * j + 64, j, 0:N], OP.mult)
            for hp in range(KC):
                mid.append(lambda hp=hp: mult_pair(hp))

            tail = []
            stats = {}

            def proj_step(co):
                ps = psG.tile([128, 1024], F32T, tag="ps")
                for p in range(3):
                    for (lo, w) in SPL_N:
                        nc.tensor.matmul(
                            ps[:, lo:lo + w],
                            lhsT=wp_sb[:, 2 * p:2 * p + 2,
                                       co * 128:co * 128 + 128],
                            rhs=onorm[:, 2 * p:2 * p + 2, lo:lo + w],
                            start=(p == 0), stop=(p == 2), perf_mode=DR)
                xb = ckp.tile([128, PW], F32T, tag="xb")
                nc.sync.dma_start(
                    xb[:, 0:N], xbT_d.ap()[b][co * 128:(co + 1) * 128, 0:N])
                x2t = ckp.tile([128, PW], F32T, tag="x2t")
                nc.vector.tensor_tensor(
                    x2t[:, 0:N], ps[:, 0:N], xb[:, 0:N], OP.add)
                nc.sync.dma_start(
                    x2_d.ap()[b][co * 128:(co + 1) * 128, 0:N], x2t[:, 0:N])
                x2b = ckp.tile([128, PW], BF16T, tag="x2b")
                nc.gpsimd.tensor_copy(x2b[:, 0:N], x2t[:, 0:N])
                sq = ckp.tile([128, PW], BF16T, tag="sq")
                nc.vector.tensor_tensor(
                    sq[:, 0:N], x2b[:, 0:N], x2b[:, 0:N], OP.mult)
                if co == 0:
                    stats['s'] = psA.tile([128, 1024], F32T, tag="ps",
                                          name="stat_s")
                    stats['q'] = psA.tile([128, 1024], F32T, tag="ps",
                                          name="stat_q")
                for (lo, w) in SPL_N:
                    nc.tensor.matmul(
                        stats['s'][0:1, lo:lo + w], lhsT=ones_sb[:, 0:1],
                        rhs=x2b[:, lo:lo + w],
                        start=(co == 0), stop=(co == KC - 1))
                    nc.tensor.matmul(
                        stats['q'][0:1, lo:lo + w], lhsT=ones_sb[:, 0:1],
                        rhs=sq[:, lo:lo + w],
                        start=(co == 0), stop=(co == KC - 1))
            for co in range(KC):
                tail.append(lambda co=co: proj_step(co))

            def rstd2_step():
                ra = denp.tile([1, PW], F32T, tag="ra")
                rb2 = denp.tile([1, PW], F32T, tag="rb2")
                nc.vector.tensor_scalar(
                    out=ra[:, 0:N], in0=stats['s'][0:1, 0:N],
                    scalar1=1.0 / C, scalar2=None, op0=OP.mult)
                nc.vector.tensor_tensor(                  # rb2 = mean^2
                    rb2[:, 0:N], ra[:, 0:N], ra[:, 0:N], OP.mult)
                nc.vector.scalar_tensor_tensor(           # ra = var
                    out=ra[:, 0:N], in0=stats['q'][0:1, 0:N], scalar=1.0 / C,
                    in1=rb2[:, 0:N], op0=OP.mult, op1=OP.subtract)
                nc.scalar.activation(rb2[:, 0:N], ra[:, 0:N], AF.Ln,
                                     bias=eps_sb[0:1, 0:1])
                nc.scalar.activation(ra[:, 0:N], rb2[:, 0:N], AF.Exp,
                                     scale=-0.5)
                nc.sync.dma_start(
                    rsd2_d.ap()[b][0:N].rearrange("(o c) -> o c", o=1),
                    ra[:, 0:N])
            tail.append(rstd2_step)

            xh2 = xh2p.tile([128, KC, PW], F8T, tag="xh2")
            fc['xh2'] = xh2

            def xh2_step(co):
                if co == 0:
                    r2bc = rowp.tile([128, PW], F32T, tag="r2bc")
                    src = rsd2_d.ap()[b][0:N]
                    nc.sync.dma_start(r2bc[:, 0:N], bass.AP(
                        tensor=src.tensor, offset=src.offset,
                        ap=[[0, 128]] + list(src.ap)))
                    stats['r2bc'] = r2bc
                x2r = ckm.tile([128, PW], F32T, tag="x2r")
                nc.sync.dma_start(
                    x2r[:, 0:N],
                    x2_d.ap()[b][co * 128:(co + 1) * 128, 0:N])
                nc.vector.tensor_tensor(
                    xh2[:, co, 0:N], x2r[:, 0:N], stats['r2bc'][:, 0:N],
                    OP.mult)
            for co in range(KC):
                tail.append(lambda co=co: xh2_step(co))
            return mid, tail

        # ---------------- MLP phase for one item ----------------
        def mlp_phase(b, fc):
            xh2 = fc['xh2']
            mt = mtp.tile([128, MC, PW], BF16T, tag="mt")
            w1ts = {}

            def load_w1(mc):
                t = w1tp.tile([128, KC, 128], F8T, tag="w1t")
                nc.sync.dma_start(
                    t[:], w1_d.ap().rearrange("(k p) c -> p k c", p=128)
                    [:, :, mc * 128:(mc + 1) * 128])
                w1ts[mc] = t

            load_w1(0)
            pspools = [psA, psA, psG]
            for mc in range(MC):
                if mc + 1 < MC:
                    load_w1(mc + 1)
                ps = pspools[mc % 3].tile([128, 1024], F32T, tag="ps")
                for p in range(3):
                    for (lo, w) in SPL_N:
                        nc.tensor.matmul(
                            ps[:, lo:lo + w],
                            lhsT=w1ts[mc][:, 2 * p:2 * p + 2, :],
                            rhs=xh2[:, 2 * p:2 * p + 2, lo:lo + w],
                            start=(p == 0), stop=(p == 2), perf_mode=DR)
                nc.scalar.activation(mt[:, mc, 0:N], ps[:, 0:N], GELU_AF,
                                     bias=bfc1_sb[:, mc:mc + 1])
                del w1ts[mc]

            w2ts = {}
            x2rs = {}

            def load_w2(co):
                t = w2tp.tile([128, MC, 128], BF16T, tag="w2t")
                nc.sync.dma_start(
                    t[:], w2_d.ap().rearrange("(k p) c -> p k c", p=128)
                    [:, :, co * 128:(co + 1) * 128])
                w2ts[co] = t
                xr = ckm.tile([128, PW], F32T, tag="x2r")
                nc.sync.dma_start(
                    xr[:, 0:N],
                    x2_d.ap()[b][co * 128:(co + 1) * 128, 0:N])
                x2rs[co] = xr

            load_w2(0)
            for co in range(KC):
                if co + 1 < KC:
                    load_w2(co + 1)
                ps = pspools[co % 3].tile([128, 1024], F32T, tag="ps")
                for k in range(MC):
                    for (lo, w) in SPL_N:
                        nc.tensor.matmul(
                            ps[:, lo:lo + w],
                            lhsT=w2ts[co][:, k, :],
                            rhs=mt[:, k, lo:lo + w],
                            start=(k == 0), stop=(k == MC - 1))
                ot = ckm.tile([128, PW], F32T, tag="ot")
                nc.vector.scalar_tensor_tensor(
                    out=ot[:, 0:N], in0=ps[:, 0:N],
                    scalar=bfc2_sb[:, co:co + 1], in1=x2rs[co][:, 0:N],
                    op0=OP.add, op1=OP.add)
                nc.sync.dma_start(
                    out_d.ap()[b][co * 128:(co + 1) * 128, 0:N], ot[:, 0:N])
                del w2ts[co], x2rs[co]

        # ---------------- orchestration ----------------
        def interleave(a, bl):
            out = []
            ia = ib = 0
            while ia < len(a) or ib < len(bl):
                if ib < len(bl):
                    out.append(bl[ib]); ib += 1
                if ia < len(a):
                    out.append(a[ia]); ia += 1
            return out

        q_steps, q_cell = qkv_steps(0)
        for s in q_steps:
            s()
        cells = {0: q_cell}
        fin_prev = None
        for b in range(BPC):
            sec_mid = []
            if b + 1 < BPC:
                qs, qc = qkv_steps(b + 1)
                cells[b + 1] = qc
                sec_mid = qs
            if fin_prev is not None:
                fmid, ftail = finalize_steps(fin_prev)
                sec_mid = interleave(fmid, sec_mid)
                sec_tail = ftail
            else:
                sec_tail = []
            fcell = attn_phase(b, cells[b], sec_mid, sec_tail)
            if fin_prev is not None:
                mlp_phase(fin_prev['b'], fin_prev)
            fin_prev = fcell
        fmid, ftail = finalize_steps(fin_prev)
        for s in fmid:
            s()
        for s in ftail:
            s()
        mlp_phase(BPC - 1, fin_prev)


def host_prep(inputs):
    """Fold LN gamma/means/scale into weights; channel-major layouts."""
    x = np.asarray(inputs['x'], F32)
    qkv_w = np.asarray(inputs['qkv_w'], F32)
    g1 = np.asarray(inputs['norm1_g'], F32)
    b1 = np.asarray(inputs['norm1_b'], F32)
    q_bias = np.asarray(inputs['q_bias'], F32)
    v_bias = np.asarray(inputs['v_bias'], F32)
    rpb_table = np.asarray(inputs['rpb_table'], F32)
    rel_index = np.asarray(inputs['rel_index'])
    proj_w = np.asarray(inputs['proj_w'], F32)
    proj_b = np.asarray(inputs['proj_b'], F32)
    g2 = np.asarray(inputs['norm2_g'], F32)
    b2 = np.asarray(inputs['norm2_b'], F32)
    fc1_w = np.asarray(inputs['fc1_w'], F32)
    fc1_b = np.asarray(inputs['fc1_b'], F32)
    fc2_w = np.asarray(inputs['fc2_w'], F32)
    fc2_b = np.asarray(inputs['fc2_b'], F32)

    assert np.allclose(b1, 0) and np.allclose(q_bias, 0) \
        and np.allclose(v_bias, 0) and np.allclose(b2, 0), \
        "zero-bias fast path only"

    Wq = qkv_w[0:C] * g1[None, :] * SCALE
    Wk = qkv_w[C:2 * C] * g1[None, :]
    Wv = qkv_w[2 * C:] * g1[None, :]
    W1 = fc1_w * g2[None, :]
    # rank-1 LN-mean fold
    Wq = Wq - Wq.mean(1, keepdims=True)
    Wk = Wk - Wk.mean(1, keepdims=True)
    Wv = Wv - Wv.mean(1, keepdims=True)
    W1 = W1 - W1.mean(1, keepdims=True)

    wqk8 = np.ascontiguousarray(np.concatenate([Wq, Wk], 0).T).astype(F8)
    wv8 = np.ascontiguousarray(Wv.T).astype(F8)
    wp8 = np.ascontiguousarray(proj_w.T).astype(F8)
    w18 = np.ascontiguousarray(W1.T).astype(F8)
    w2T = np.ascontiguousarray(fc2_w.T).astype(BF16)
    bias_fc1 = fc1_b.astype(F32)
    bias_fc2 = fc2_b.astype(F32)

    rpb = rpb_table[rel_index]                     # [N, N, NH]
    rpbT = rpb.transpose(2, 1, 0)                  # [H, m(key), n(query)]
    rpb8 = np.full((KC, 128, 2, MCHUNK, PW), -30.0, F32)
    for hp in range(KC):
        for j in range(2):
            for mc in range(MCHUNK):
                m0 = mc * 128
                mw = min(128, N - m0)
                rpb8[hp, 0:mw, j, mc, 0:N] = rpbT[2 * hp + j, m0:m0 + mw, :]
    rpb8 = rpb8.astype(F8)

    xpad = np.zeros((B, NPAD, C), F32)
    xpad[:, :N, :] = x
    mean1 = xpad.mean(2)
    var1 = xpad.var(2)
    rstd1 = (1.0 / np.sqrt(var1 + EPS)).astype(F32)        # [B, NPAD]
    del mean1
    xT8 = np.ascontiguousarray(xpad.transpose(0, 2, 1)).astype(F8)
    xbT = np.ascontiguousarray(
        (xpad + proj_b[None, None, :]).transpose(0, 2, 1)).astype(F32)

    shared = dict(wqk8=wqk8, wv8=wv8, wp8=wp8, w18=w18, w2T=w2T,
                  bias_fc1=bias_fc1, bias_fc2=bias_fc2, rpb8=rpb8)
    in_maps = []
    for core in range(N_CORES):
        sl = slice(core * BPC, (core + 1) * BPC)
        m = dict(shared)
        m['xT8'] = np.ascontiguousarray(xT8[sl])
        m['xbT'] = np.ascontiguousarray(xbT[sl])
        m['rstd1'] = np.ascontiguousarray(rstd1[sl])
        in_maps.append(m)
    return in_maps


def build_bass():
    nc = bacc.Bacc("TRN2", target_bir_lowering=False, debug=False,
                   num_devices=N_CORES)
    build_program(nc)
    nc.compile()
    return nc


def gather_output(results):
    out = np.zeros((B, N, C), F32)
    for core in range(N_CORES):
        o = results[core]["outT"]                   # [BPC, C, NPAD]
        out[core * BPC:(core + 1) * BPC] = o.transpose(0, 2, 1)[:, :N, :]
    return out


def kernel(**inputs):
    in_maps = host_prep(inputs)
    nc = build_bass()
    res = bass_utils.run_bass_kernel_spmd(nc, in_maps,
                                          core_ids=list(range(N_CORES)))
    return gather_output(res.results)


# revision 49
# speedup vs baseline: 1.2179x; 1.2179x over previous
"""Trainium2 Bass kernel for a ViT attention block (LN->MHA+relpos->LN->MLP).

Contract: kernel(**inputs) takes the FULL unsharded inputs, shards batch
across 8 NeuronCores (4 items per core), runs one SPMD Bass program, and
gathers the full [32, 577, 768] fp32 output.

v2 design (channel-major):
- All activations flow as [channel(partition), token(free)] slabs; the host
  ships x pre-transposed, so no DMA-xbar transposes on device.
- LayerNorm means are folded into the next matmul's weights as a rank-1
  correction (W' = W - rowmean(W)); LN1's rstd is computed on the host
  (it depends only on the input x), LN2's rstd via ones-matmul token sums.
- fp8(e4m3) DoubleRow matmuls for qkv, proj, PV, and fc1 (2x PE columns);
  fc2 stays bf16 for accuracy; S stays bf16.
- S matmuls are K=64 row-tiled: head pairs land on PE row-strips (0,0) and
  (64,0) and run concurrently.
- Relative-position bias is added into the S PSUM tile by the Pool engine
  (nc.gpsimd), not by identity matmuls; exp runs on Act straight out of
  PSUM into fp8.
- P^T @ [v|1] (fp8 DoubleRow over 6 m-chunk pairs) yields O^T plus the
  softmax denominator via an ones-column in the v slab.
- fc1/fc2 weights are streamed from DRAM per chunk (SBUF pressure).
"""

import sys

if '/opt/trn_rl_repo' not in sys.path:
    sys.path.insert(0, '/opt/trn_rl_repo')

from contextlib import ExitStack

import numpy as np
import ml_dtypes

import concourse.bass as bass  # noqa: F401
import concourse.tile as tile
import concourse.mybir as mybir
from concourse import bacc, bass_utils

BF16 = ml_dtypes.bfloat16
F8 = ml_dtypes.float8_e4m3   # TRN fp8e4 (e4m3, max 240)
F32 = np.float32

B = 32
N = 577
C = 768
NH = 12
HD = 64
MLP = 3072
EPS = 1e-6
SCALE = HD ** (-0.5)

N_CORES = 8
BPC = B // N_CORES          # 4 batch items per core
NPAD = 640                  # per-item padded token count (5 * 128)
KC = C // 128               # 6 contraction chunks for dim 768
MC = MLP // 128             # 24 chunks for MLP dim
MCHUNK = 5                  # m-chunks covering 577 tokens (4*128 + 65)
PW = 592                    # padded 577 (stride % 16 == 0)
VROW = 864                  # v slab row: 12 heads * 66 + pad

F32T = mybir.dt.float32
BF16T = mybir.dt.bfloat16
F8T = mybir.dt.float8e4
AF = mybir.ActivationFunctionType
OP = mybir.AluOpType
DR = mybir.MatmulPerfMode.DoubleRow

SPL_N = [(0, 512), (512, 65)]    # 577-wide streams
SPL_P = [(0, 512), (512, 128)]   # 640-wide streams

GELU_AF = AF.Gelu  # sim_test swaps this (CoreSim lacks Gelu); HW uses Gelu
ZERO_ALL_SLABS = False  # sim-only: defeat the pool-slot zero-persistence


def build_program(nc):
    dt = mybir.dt

    xT_d = nc.dram_tensor("xT8", [BPC, C, NPAD], dt.float8e4, kind="ExternalInput")
    xbT_d = nc.dram_tensor("xbT", [BPC, C, NPAD], dt.float32, kind="ExternalInput")
    rstd1_d = nc.dram_tensor("rstd1", [BPC, NPAD], dt.float32, kind="ExternalInput")
    wqk_d = nc.dram_tensor("wqk8", [C, 2 * C], dt.float8e4, kind="ExternalInput")
    wv_d = nc.dram_tensor("wv8", [C, C], dt.float8e4, kind="ExternalInput")
    wp_d = nc.dram_tensor("wp8", [C, C], dt.float8e4, kind="ExternalInput")
    w1_d = nc.dram_tensor("w18", [C, MLP], dt.float8e4, kind="ExternalInput")
    w2_d = nc.dram_tensor("w2T", [MLP, C], dt.bfloat16, kind="ExternalInput")
    b1_d = nc.dram_tensor("bias_fc1", [MLP], dt.float32, kind="ExternalInput")
    b2_d = nc.dram_tensor("bias_fc2", [C], dt.float32, kind="ExternalInput")
    rpb_d = nc.dram_tensor("rpb8", [KC, 128, 2, MCHUNK, PW], dt.float8e4,
                           kind="ExternalInput")
    id_d = nc.dram_tensor("ident8", [128, 128], dt.float8e4,
                          kind="ExternalInput")
    out_d = nc.dram_tensor("outT", [BPC, C, NPAD], dt.float32,
                           kind="ExternalOutput")

    x2_d = nc.dram_tensor("x2T_scratch", [BPC, C, NPAD], dt.float32)
    rec_d = nc.dram_tensor("rec_scratch", [BPC, NH, PW], dt.float32)
    rsd2_d = nc.dram_tensor("rstd2_scratch", [BPC, NPAD], dt.float32)

    with tile.TileContext(nc) as tc, ExitStack() as ctx:
        psA = ctx.enter_context(tc.tile_pool(name="psA", bufs=2, space="PSUM"))
        psPV = ctx.enter_context(tc.tile_pool(name="psPV", bufs=1, space="PSUM"))
        psG = ctx.enter_context(tc.tile_pool(name="psG", bufs=1, space="PSUM"))

        persist = ctx.enter_context(tc.tile_pool(name="persist", bufs=1))
        wqk_sb = persist.tile([128, KC, 2 * C], F8T, tag="wqk")
        nc.sync.dma_start(wqk_sb[:], wqk_d.ap().rearrange("(k p) c -> p k c", p=128))
        wv_sb = persist.tile([128, KC, C], F8T, tag="wv")
        nc.sync.dma_start(wv_sb[:], wv_d.ap().rearrange("(k p) c -> p k c", p=128))
        wp_sb = persist.tile([128, KC, C], F8T, tag="wp")
        nc.sync.dma_start(wp_sb[:], wp_d.ap().rearrange("(k p) c -> p k c", p=128))
        ones_sb = persist.tile([128, 1], BF16T, tag="ones")
        nc.vector.memset(ones_sb[:], 1.0)
        id_sb = persist.tile([128, 128], F8T, tag="ident")
        nc.sync.dma_start(id_sb[:], id_d.ap())
        eps_sb = persist.tile([128, 1], F32T, tag="eps")
        nc.vector.memset(eps_sb[:], EPS)
        bfc1_sb = persist.tile([128, MC], F32T, tag="bfc1")
        nc.sync.dma_start(bfc1_sb[:], b1_d.ap().rearrange("(m p) -> p m", p=128))
        bfc2_sb = persist.tile([128, KC], F32T, tag="bfc2")
        nc.sync.dma_start(bfc2_sb[:], b2_d.ap().rearrange("(m p) -> p m", p=128))

        xtp = ctx.enter_context(tc.tile_pool(name="xtp", bufs=2))
        qkp = ctx.enter_context(tc.tile_pool(name="qkp", bufs=2))
        kzp = ctx.enter_context(tc.tile_pool(name="kzp", bufs=2))
        v8p = ctx.enter_context(tc.tile_pool(name="v8p", bufs=2))
        ptp = ctx.enter_context(tc.tile_pool(name="ptp", bufs=2))
        rpbp = ctx.enter_context(tc.tile_pool(name="rpbp", bufs=2))
        osbp = ctx.enter_context(tc.tile_pool(name="osbp", bufs=2))
        onp = ctx.enter_context(tc.tile_pool(name="onp", bufs=2))
        xh2p = ctx.enter_context(tc.tile_pool(name="xh2p", bufs=2))
        mtp = ctx.enter_context(tc.tile_pool(name="mtp", bufs=1))
        w1tp = ctx.enter_context(tc.tile_pool(name="w1tp", bufs=2))
        w2tp = ctx.enter_context(tc.tile_pool(name="w2tp", bufs=2))
        ckp = ctx.enter_context(tc.tile_pool(name="ckp", bufs=2))
        ckm = ctx.enter_context(tc.tile_pool(name="ckm", bufs=2))
        rowp = ctx.enter_context(tc.tile_pool(name="rowp", bufs=2))
        rbp = ctx.enter_context(tc.tile_pool(name="rbp", bufs=2))
        smallp = ctx.enter_context(tc.tile_pool(name="smallp", bufs=2))
        denp = ctx.enter_context(tc.tile_pool(name="denp", bufs=1))

        # pool-slot zero-persistence counters (one per pool; slots alternate
        # and the zeroed regions are never overwritten by data)
        v8_cnt = [0]
        pt_cnt = [0]
        on_cnt = [0]

        # ---------------- qkv production for one item ----------------
        def qkv_steps(b):
            cell = {}

            def alloc_step():
                xt = xtp.tile([128, KC, NPAD], F8T, tag="xT")
                nc.sync.dma_start(
                    xt[:], xT_d.ap()[b].rearrange("(k p) t -> p k t", p=128))
                rrow = rowp.tile([128, NPAD], F32T, tag="rrow")
                src = rstd1_d.ap()[b]
                nc.sync.dma_start(rrow[:], bass.AP(
                    tensor=src.tensor, offset=src.offset,
                    ap=[[0, 128]] + list(src.ap)))
                rcol = smallp.tile([128, MCHUNK], F32T, tag="rcol")
                nc.sync.dma_start(
                    rcol[:], rstd1_d.ap()[b].rearrange("(c p) -> p c", p=128))
                qkT = qkp.tile([128, KC, PW], BF16T, tag="qkT")
                kz = kzp.tile([128, KC, PW], BF16T, tag="kz")
                v8 = v8p.tile([128, MCHUNK, VROW], F8T, tag="v8")
                if v8_cnt[0] < 2 or ZERO_ALL_SLABS:
                    v8_cnt[0] += 1
                    nc.vector.memset(v8[:], 0.0)
                    # ones columns (softmax denominator)
                    nc.vector.memset(
                        v8[:, :, 0:NH * 66].rearrange(
                            "p m (h e) -> p m h e", e=66)[:, :, :, 64:66], 1.0)
                cell.update(xT=xt, rrow=rrow, rcol=rcol, qkT=qkT, kz=kz, v8=v8)

            def qk_step(hp, which):
                ps = psG.tile([128, 1024], F32T, tag="ps")
                c0 = which * C + hp * 128
                for p in range(3):
                    for (lo, w) in SPL_N:
                        nc.tensor.matmul(
                            ps[:, lo:lo + w],
                            lhsT=wqk_sb[:, 2 * p:2 * p + 2, c0:c0 + 128],
                            rhs=cell['xT'][:, 2 * p:2 * p + 2, lo:lo + w],
                            start=(p == 0), stop=(p == 2), perf_mode=DR)
                dest = cell['qkT'] if which == 0 else cell['kz']
                nc.vector.tensor_tensor(
                    dest[:, hp, 0:N], ps[:, 0:N], cell['rrow'][:, 0:N], OP.mult)

            def v_step(mc):
                mw = 128 if mc < MCHUNK - 1 else N - 512
                ps = psG.tile([128, 1024], F32T, tag="ps")
                for p in range(3):
                    for (lo, w) in [(0, 512), (512, 256)]:
                        nc.tensor.matmul(
                            ps[0:mw, lo:lo + w],
                            lhsT=cell['xT'][:, 2 * p:2 * p + 2,
                                            mc * 128:mc * 128 + mw],
                            rhs=wv_sb[:, 2 * p:2 * p + 2, lo:lo + w],
                            start=(p == 0), stop=(p == 2), perf_mode=DR)
                nc.vector.tensor_scalar(
                    out=cell['v8'][0:mw, mc, 0:NH * 66].rearrange(
                        "p (h e) -> p h e", e=66)[:, :, 0:64],
                    in0=ps[0:mw, 0:C].rearrange("p (h e) -> p h e", h=NH),
                    scalar1=cell['rcol'][0:mw, mc:mc + 1], scalar2=None,
                    op0=OP.mult)

            steps = [alloc_step]
            for hp in range(KC):
                steps.append(lambda hp=hp: qk_step(hp, 0))
                steps.append(lambda hp=hp: qk_step(hp, 1))
            for mc in range(MCHUNK):
                steps.append(lambda mc=mc: v_step(mc))
            return steps, cell

        # ---------------- PV drain context ----------------
        class PvCtx:
            def __init__(self, pt, v8, h, osb, den12):
                self.pt, self.v8, self.h = pt, v8, h
                self.osb, self.den12 = osb, den12
                self.pv = None
                self.mms = [(p, lo, w) for p in range(3) for (lo, w) in SPL_N]
                self.pos = 0

            def drain(self, k):
                if self.pv is None:
                    self.pv = psPV.tile([128, 1024], F32T, tag="pv")
                h, j = self.h, self.h % 2
                w0 = h * 66
                mtail = N - 512
                end = min(self.pos + k, len(self.mms))
                for (p, lo, w) in self.mms[self.pos:end]:
                    if p < 2:
                        nc.tensor.matmul(
                            self.pv[:, lo:lo + w],
                            lhsT=self.v8[:, 2 * p:2 * p + 2, w0:w0 + 128],
                            rhs=self.pt[:, j, 2 * p:2 * p + 2, lo:lo + w],
                            start=(p == 0), stop=False, perf_mode=DR)
                    else:
                        nc.tensor.matmul(
                            self.pv[:, lo:lo + w],
                            lhsT=self.v8[0:mtail, 4, w0:w0 + 128],
                            rhs=self.pt[0:mtail, j, 4, lo:lo + w],
                            start=False, stop=True)
                self.pos = end
                if self.pos == len(self.mms):
                    hp = h // 2
                    dd = smallp.tile([1, PW], F32T, tag="dd")
                    nc.scalar.activation(dd[:, 0:N], self.pv[64:65, 0:N],
                                         AF.Identity)
                    nc.sync.dma_start(
                        self.den12[h:h + 1, 0:N], dd[:, 0:N])
                    nc.vector.tensor_copy(
                        self.osb[64 * j:64 * j + 64, hp, 0:N],
                        self.pv[0:64, 0:N])
                    self.pv = None
                    return True
                return False

            def finish(self):
                while self.pos < len(self.mms):
                    self.drain(6)

        # ---------------- attention phase for one item ----------------
        def attn_phase(b, cell, sec_mid, sec_tail):
            qkT, kz, v8 = cell['qkT'], cell['kz'], cell['v8']
            osb = osbp.tile([128, KC, PW], F8T, tag="osb")
            den12 = denp.tile([NH, PW], F32T, tag="den12")
            rts = {}

            def load_rpb(hp):
                rt = rpbp.tile([128, 2, MCHUNK, PW], F8T, tag="rpb")
                nc.sync.dma_start(rt[:], rpb_d.ap()[hp])
                rts[hp] = rt

            load_rpb(0)
            pend = []
            sec_i = [0]

            def run_sec(n):
                for _ in range(n):
                    if sec_i[0] < len(sec_mid):
                        sec_mid[sec_i[0]]()
                        sec_i[0] += 1

            for hp in range(KC):
                if hp + 1 < KC:
                    load_rpb(hp + 1)
                pt = ptp.tile([128, 2, MCHUNK, PW], F8T, tag="pt")
                for mc in range(MCHUNK):
                    mw = 128 if mc < MCHUNK - 1 else N - 512
                    m0 = mc * 128
                    sp2 = []
                    for j in (0, 1):  # row-tiled K=64 pair, concurrent
                        sps = psA.tile([128, 1024], F32T, tag="ps")
                        sp2.append(sps)
                        b0 = 64 * j
                        for (lo, w) in SPL_N:
                            nc.tensor.matmul(
                                sps[0:mw, lo:lo + w],
                                lhsT=kz[b0:b0 + 64, hp, m0:m0 + mw],
                                rhs=qkT[b0:b0 + 64, hp, lo:lo + w],
                                start=True, stop=False)
                    for j in (0, 1):  # rpb accumulate via fp8 identity
                        sps = sp2[j]
                        for (lo, w) in SPL_N:
                            nc.tensor.matmul(
                                sps[0:mw, lo:lo + w],
                                lhsT=id_sb[0:mw, 0:mw],
                                rhs=rts[hp][0:mw, j, mc, lo:lo + w],
                                start=False, stop=True)
                        nc.scalar.activation(
                            pt[0:mw, j, mc, 0:N], sps[0:mw, 0:N], AF.Exp)
                    if pend:
                        if pend[0].drain(2):
                            pend.pop(0)
                    run_sec(1)
                pend.append(PvCtx(pt, v8, 2 * hp, osb, den12))
                pend.append(PvCtx(pt, v8, 2 * hp + 1, osb, den12))
                while len(pend) > 2:
                    if pend[0].drain(6):
                        pend.pop(0)
            while pend:
                if pend[0].drain(6):
                    pend.pop(0)
                run_sec(1)
            rec12 = denp.tile([NH, PW], F32T, tag="rec12")
            nc.vector.reciprocal(rec12[:, 0:N], den12[:, 0:N])
            nc.sync.dma_start(rec_d.ap()[b][:, 0:N], rec12[:, 0:N])
            while sec_i[0] < len(sec_mid):
                sec_mid[sec_i[0]]()
                sec_i[0] += 1
            for s in sec_tail:
                s()
            return dict(osb=osb, b=b)

        # ---------------- finalize (runs inside next attn phase) -------
        def finalize_steps(fc):
            b, osb = fc['b'], fc['osb']
            onorm = onp.tile([128, KC, PW], F8T, tag="onorm")
            mid = []

            def mult_pair(hp):
                rb = rbp.tile([128, 2, PW], F32T, tag="rb")
                src = rec_d.ap()[b][2 * hp:2 * hp + 2, 0:N]
                nc.sync.dma_start(rb[:, :, 0:N], bass.AP(
                    tensor=src.tensor, offset=src.offset,
                    ap=[[0, 128]] + list(src.ap)))
                for j in (0, 1):
                    nc.vector.tensor_tensor(
                        onorm[64 * j:64 * j + 64, hp, 0:N],
                        osb[64 * j:64 * j + 64, hp, 0:N],
                        rb[64 * j:64 * j + 64, j, 0:N], OP.mult)
            for hp in range(KC):
                mid.append(lambda hp=hp: mult_pair(hp))

            tail = []
            stats = {}

            def proj_step(co):
                ps = psG.tile([128, 1024], F32T, tag="ps")
                for p in range(3):
                    for (lo, w) in SPL_N:
                        nc.tensor.matmul(
                            ps[:, lo:lo + w],
                            lhsT=wp_sb[:, 2 * p:2 * p + 2,
                                       co * 128:co * 128 + 128],
                            rhs=onorm[:, 2 * p:2 * p + 2, lo:lo + w],
                            start=(p == 0), stop=(p == 2), perf_mode=DR)
                xb = ckp.tile([128, PW], F32T, tag="xb")
                nc.sync.dma_start(
                    xb[:, 0:N], xbT_d.ap()[b][co * 128:(co + 1) * 128, 0:N])
                x2t = ckp.tile([128, PW], F32T, tag="x2t")
                nc.vector.tensor_tensor(
                    x2t[:, 0:N], ps[:, 0:N], xb[:, 0:N], OP.add)
                nc.sync.dma_start(
                    x2_d.ap()[b][co * 128:(co + 1) * 128, 0:N], x2t[:, 0:N])
                x2b = ckp.tile([128, PW], BF16T, tag="x2b")
                nc.vector.tensor_copy(x2b[:, 0:N], x2t[:, 0:N])
                sq = ckp.tile([128, PW], BF16T, tag="sq")
                nc.vector.tensor_tensor(
                    sq[:, 0:N], x2b[:, 0:N], x2b[:, 0:N], OP.mult)
                if co == 0:
                    stats['s'] = psA.tile([128, 1024], F32T, tag="ps",
                                          name="stat_s")
                    stats['q'] = psA.tile([128, 1024], F32T, tag="ps",
                                          name="stat_q")
                for (lo, w) in SPL_N:
                    nc.tensor.matmul(
                        stats['s'][0:1, lo:lo + w], lhsT=ones_sb[:, 0:1],
                        rhs=x2b[:, lo:lo + w],
                        start=(co == 0), stop=(co == KC - 1))
                    nc.tensor.matmul(
                        stats['q'][0:1, lo:lo + w], lhsT=ones_sb[:, 0:1],
                        rhs=sq[:, lo:lo + w],
                        start=(co == 0), stop=(co == KC - 1))
            for co in range(KC):
                tail.append(lambda co=co: proj_step(co))

            def rstd2_step():
                ra = denp.tile([1, PW], F32T, tag="ra")
                rb2 = denp.tile([1, PW], F32T, tag="rb2")
                nc.vector.tensor_scalar(
                    out=ra[:, 0:N], in0=stats['s'][0:1, 0:N],
                    scalar1=1.0 / C, scalar2=None, op0=OP.mult)
                nc.vector.tensor_tensor(                  # rb2 = mean^2
                    rb2[:, 0:N], ra[:, 0:N], ra[:, 0:N], OP.mult)
                nc.vector.scalar_tensor_tensor(           # ra = var
                    out=ra[:, 0:N], in0=stats['q'][0:1, 0:N], scalar=1.0 / C,
                    in1=rb2[:, 0:N], op0=OP.mult, op1=OP.subtract)
                nc.scalar.activation(rb2[:, 0:N], ra[:, 0:N], AF.Ln,
                                     bias=eps_sb[0:1, 0:1])
                nc.scalar.activation(ra[:, 0:N], rb2[:, 0:N], AF.Exp,
                                     scale=-0.5)
                nc.sync.dma_start(
                    rsd2_d.ap()[b][0:N].rearrange("(o c) -> o c", o=1),
                    ra[:, 0:N])
            tail.append(rstd2_step)

            xh2 = xh2p.tile([128, KC, PW], F8T, tag="xh2")
            fc['xh2'] = xh2

            def xh2_step(co):
                if co == 0:
                    r2bc = rowp.tile([128, PW], F32T, tag="r2bc")
                    src = rsd2_d.ap()[b][0:N]
                    nc.sync.dma_start(r2bc[:, 0:N], bass.AP(
                        tensor=src.tensor, offset=src.offset,
                        ap=[[0, 128]] + list(src.ap)))
                    stats['r2bc'] = r2bc
                x2r = ckm.tile([128, PW], F32T, tag="x2r")
                nc.sync.dma_start(
                    x2r[:, 0:N],
                    x2_d.ap()[b][co * 128:(co + 1) * 128, 0:N])
                nc.vector.tensor_tensor(
                    xh2[:, co, 0:N], x2r[:, 0:N], stats['r2bc'][:, 0:N],
                    OP.mult)
            for co in range(KC):
                tail.append(lambda co=co: xh2_step(co))
            return mid, tail

        # ---------------- MLP phase for one item ----------------
        def mlp_phase(b, fc):
            xh2 = fc['xh2']
            mt = mtp.tile([128, MC, PW], BF16T, tag="mt")
            w1ts = {}

            def load_w1(mc):
                t = w1tp.tile([128, KC, 128], F8T, tag="w1t")
                nc.sync.dma_start(
                    t[:], w1_d.ap().rearrange("(k p) c -> p k c", p=128)
                    [:, :, mc * 128:(mc + 1) * 128])
                w1ts[mc] = t

            load_w1(0)
            pspools = [psA, psA, psG]
            for mc in range(MC):
                if mc + 1 < MC:
                    load_w1(mc + 1)
                ps = pspools[mc % 3].tile([128, 1024], F32T, tag="ps")
                for p in range(3):
                    for (lo, w) in SPL_N:
                        nc.tensor.matmul(
                            ps[:, lo:lo + w],
                            lhsT=w1ts[mc][:, 2 * p:2 * p + 2, :],
                            rhs=xh2[:, 2 * p:2 * p + 2, lo:lo + w],
                            start=(p == 0), stop=(p == 2), perf_mode=DR)
                nc.scalar.activation(mt[:, mc, 0:N], ps[:, 0:N], GELU_AF,
                                     bias=bfc1_sb[:, mc:mc + 1])
                del w1ts[mc]

            w2ts = {}
            x2rs = {}

            def load_w2(co):
                t = w2tp.tile([128, MC, 128], BF16T, tag="w2t")
                nc.sync.dma_start(
                    t[:], w2_d.ap().rearrange("(k p) c -> p k c", p=128)
                    [:, :, co * 128:(co + 1) * 128])
                w2ts[co] = t
                xr = ckm.tile([128, PW], F32T, tag="x2r")
                nc.sync.dma_start(
                    xr[:, 0:N],
                    x2_d.ap()[b][co * 128:(co + 1) * 128, 0:N])
                x2rs[co] = xr

            load_w2(0)
            for co in range(KC):
                if co + 1 < KC:
                    load_w2(co + 1)
                ps = pspools[co % 3].tile([128, 1024], F32T, tag="ps")
                for k in range(MC):
                    for (lo, w) in SPL_N:
                        nc.tensor.matmul(
                            ps[:, lo:lo + w],
                            lhsT=w2ts[co][:, k, :],
                            rhs=mt[:, k, lo:lo + w],
                            start=(k == 0), stop=(k == MC - 1))
                ot = ckm.tile([128, PW], F32T, tag="ot")
                nc.vector.scalar_tensor_tensor(
                    out=ot[:, 0:N], in0=ps[:, 0:N],
                    scalar=bfc2_sb[:, co:co + 1], in1=x2rs[co][:, 0:N],
                    op0=OP.add, op1=OP.add)
                nc.sync.dma_start(
                    out_d.ap()[b][co * 128:(co + 1) * 128, 0:N], ot[:, 0:N])
                del w2ts[co], x2rs[co]

        # ---------------- orchestration ----------------
        def interleave(a, bl):
            out = []
            ia = ib = 0
            while ia < len(a) or ib < len(bl):
                if ib < len(bl):
                    out.append(bl[ib]); ib += 1
                if ia < len(a):
                    out.append(a[ia]); ia += 1
            return out

        q_steps, q_cell = qkv_steps(0)
        for s in q_steps:
            s()
        cells = {0: q_cell}
        fin_prev = None
        for b in range(BPC):
            sec_mid = []
            if b + 1 < BPC:
                qs, qc = qkv_steps(b + 1)
                cells[b + 1] = qc
                sec_mid = qs
            if fin_prev is not None:
                fmid, ftail = finalize_steps(fin_prev)
                sec_mid = interleave(fmid, sec_mid)
                sec_tail = ftail
            else:
                sec_tail = []
            fcell = attn_phase(b, cells[b], sec_mid, sec_tail)
            if fin_prev is not None:
                mlp_phase(fin_prev['b'], fin_prev)
            fin_prev = fcell
        fmid, ftail = finalize_steps(fin_prev)
        for s in fmid:
            s()
        for s in ftail:
            s()
        mlp_phase(BPC - 1, fin_prev)


def host_prep(inputs):
    """Fold LN gamma/means/scale into weights; channel-major layouts."""
    x = np.asarray(inputs['x'], F32)
    qkv_w = np.asarray(inputs['qkv_w'], F32)
    g1 = np.asarray(inputs['norm1_g'], F32)
    b1 = np.asarray(inputs['norm1_b'], F32)
    q_bias = np.asarray(inputs['q_bias'], F32)
    v_bias = np.asarray(inputs['v_bias'], F32)
    rpb_table = np.asarray(inputs['rpb_table'], F32)
    rel_index = np.asarray(inputs['rel_index'])
    proj_w = np.asarray(inputs['proj_w'], F32)
    proj_b = np.asarray(inputs['proj_b'], F32)
    g2 = np.asarray(inputs['norm2_g'], F32)
    b2 = np.asarray(inputs['norm2_b'], F32)
    fc1_w = np.asarray(inputs['fc1_w'], F32)
    fc1_b = np.asarray(inputs['fc1_b'], F32)
    fc2_w = np.asarray(inputs['fc2_w'], F32)
    fc2_b = np.asarray(inputs['fc2_b'], F32)

    assert np.allclose(b1, 0) and np.allclose(q_bias, 0) \
        and np.allclose(v_bias, 0) and np.allclose(b2, 0), \
        "zero-bias fast path only"

    Wq = qkv_w[0:C] * g1[None, :] * SCALE
    Wk = qkv_w[C:2 * C] * g1[None, :]
    Wv = qkv_w[2 * C:] * g1[None, :]
    W1 = fc1_w * g2[None, :]
    # rank-1 LN-mean fold
    Wq = Wq - Wq.mean(1, keepdims=True)
    Wk = Wk - Wk.mean(1, keepdims=True)
    Wv = Wv - Wv.mean(1, keepdims=True)
    W1 = W1 - W1.mean(1, keepdims=True)

    wqk8 = np.ascontiguousarray(np.concatenate([Wq, Wk], 0).T).astype(F8)
    wv8 = np.ascontiguousarray(Wv.T).astype(F8)
    wp8 = np.ascontiguousarray(proj_w.T).astype(F8)
    w18 = np.ascontiguousarray(W1.T).astype(F8)
    w2T = np.ascontiguousarray(fc2_w.T).astype(BF16)
    bias_fc1 = fc1_b.astype(F32)
    bias_fc2 = fc2_b.astype(F32)

    rpb = rpb_table[rel_index]                     # [N, N, NH]
    rpbT = rpb.transpose(2, 1, 0)                  # [H, m(key), n(query)]
    rpb8 = np.full((KC, 128, 2, MCHUNK, PW), -30.0, F32)
    for hp in range(KC):
        for j in range(2):
            for mc in range(MCHUNK):
                m0 = mc * 128
                mw = min(128, N - m0)
                rpb8[hp, 0:mw, j, mc, 0:N] = rpbT[2 * hp + j, m0:m0 + mw, :]
    rpb8 = rpb8.astype(F8)

    ident8 = np.eye(128, dtype=np.float32).astype(F8)

    xpad = np.zeros((B, NPAD, C), F32)
    xpad[:, :N, :] = x
    mean1 = xpad.mean(2)
    var1 = xpad.var(2)
    rstd1 = (1.0 / np.sqrt(var1 + EPS)).astype(F32)        # [B, NPAD]
    del mean1
    xT8 = np.ascontiguousarray(xpad.transpose(0, 2, 1)).astype(F8)
    xbT = np.ascontiguousarray(
        (xpad + proj_b[None, None, :]).transpose(0, 2, 1)).astype(F32)

    shared = dict(wqk8=wqk8, wv8=wv8, wp8=wp8, w18=w18, w2T=w2T,
                  bias_fc1=bias_fc1, bias_fc2=bias_fc2, rpb8=rpb8,
                  ident8=ident8)
    in_maps = []
    for core in range(N_CORES):
        sl = slice(core * BPC, (core + 1) * BPC)
        m = dict(shared)
        m['xT8'] = np.ascontiguousarray(xT8[sl])
        m['xbT'] = np.ascontiguousarray(xbT[sl])
        m['rstd1'] = np.ascontiguousarray(rstd1[sl])
        in_maps.append(m)
    return in_maps


def build_bass():
    nc = bacc.Bacc("TRN2", target_bir_lowering=False, debug=False,
                   num_devices=N_CORES)
    build_program(nc)
    nc.compile()
    return nc


def gather_output(results):
    out = np.zeros((B, N, C), F32)
    for core in range(N_CORES):
        o = results[core]["outT"]                   # [BPC, C, NPAD]
        out[core * BPC:(core + 1) * BPC] = o.transpose(0, 2, 1)[:, :N, :]
    return out


def kernel(**inputs):
    in_maps = host_prep(inputs)
    nc = build_bass()
    res = bass_utils.run_bass_kernel_spmd(nc, in_maps,
                                          core_ids=list(range(N_CORES)))
    return gather_output(res.results)


# revision 52
# speedup vs baseline: 1.2350x; 1.0140x over previous
"""Trainium2 Bass kernel for a ViT attention block (LN->MHA+relpos->LN->MLP).

Contract: kernel(**inputs) takes the FULL unsharded inputs, shards batch
across 8 NeuronCores (4 items per core), runs one SPMD Bass program, and
gathers the full [32, 577, 768] fp32 output.

v2 design (channel-major):
- All activations flow as [channel(partition), token(free)] slabs; the host
  ships x pre-transposed, so no DMA-xbar transposes on device.
- LayerNorm means are folded into the next matmul's weights as a rank-1
  correction (W' = W - rowmean(W)); LN1's rstd is computed on the host
  (it depends only on the input x), LN2's rstd via ones-matmul token sums.
- fp8(e4m3) DoubleRow matmuls for qkv, proj, PV, and fc1 (2x PE columns);
  fc2 stays bf16 for accuracy; S stays bf16.
- S matmuls are K=64 row-tiled: head pairs land on PE row-strips (0,0) and
  (64,0) and run concurrently.
- Relative-position bias is added into the S PSUM tile by the Pool engine
  (nc.gpsimd), not by identity matmuls; exp runs on Act straight out of
  PSUM into fp8.
- P^T @ [v|1] (fp8 DoubleRow over 6 m-chunk pairs) yields O^T plus the
  softmax denominator via an ones-column in the v slab.
- fc1/fc2 weights are streamed from DRAM per chunk (SBUF pressure).
"""

import sys

if '/opt/trn_rl_repo' not in sys.path:
    sys.path.insert(0, '/opt/trn_rl_repo')

from contextlib import ExitStack

import numpy as np
import ml_dtypes

import concourse.bass as bass  # noqa: F401
import concourse.tile as tile
import concourse.mybir as mybir
from concourse import bacc, bass_utils

BF16 = ml_dtypes.bfloat16
F8 = ml_dtypes.float8_e4m3   # TRN fp8e4 (e4m3, max 240)
F32 = np.float32

B = 32
N = 577
C = 768
NH = 12
HD = 64
MLP = 3072
EPS = 1e-6
SCALE = HD ** (-0.5)

N_CORES = 8
BPC = B // N_CORES          # 4 batch items per core
NPAD = 640                  # per-item padded token count (5 * 128)
KC = C // 128               # 6 contraction chunks for dim 768
MC = MLP // 128             # 24 chunks for MLP dim
MCHUNK = 5                  # m-chunks covering 577 tokens (4*128 + 65)
PW = 592                    # padded 577 (stride % 16 == 0)
VROW = 864                  # v slab row: 12 heads * 66 + pad

F32T = mybir.dt.float32
BF16T = mybir.dt.bfloat16
F8T = mybir.dt.float8e4
AF = mybir.ActivationFunctionType
OP = mybir.AluOpType
DR = mybir.MatmulPerfMode.DoubleRow

SPL_N = [(0, 512), (512, 65)]    # 577-wide streams
SPL_P = [(0, 512), (512, 128)]   # 640-wide streams

GELU_AF = AF.Gelu  # sim_test swaps this (CoreSim lacks Gelu); HW uses Gelu
ZERO_ALL_SLABS = False  # sim-only: defeat the pool-slot zero-persistence


def build_program(nc):
    dt = mybir.dt

    xT_d = nc.dram_tensor("xT8", [BPC, C, NPAD], dt.float8e4, kind="ExternalInput")
    xbT_d = nc.dram_tensor("xbT", [BPC, C, NPAD], dt.float32, kind="ExternalInput")
    rstd1_d = nc.dram_tensor("rstd1", [BPC, NPAD], dt.float32, kind="ExternalInput")
    wqk_d = nc.dram_tensor("wqk8", [C, 2 * C], dt.float8e4, kind="ExternalInput")
    wv_d = nc.dram_tensor("wv8", [C, C], dt.float8e4, kind="ExternalInput")
    wp_d = nc.dram_tensor("wp8", [C, C], dt.float8e4, kind="ExternalInput")
    w1_d = nc.dram_tensor("w18", [C, MLP], dt.float8e4, kind="ExternalInput")
    w2_d = nc.dram_tensor("w2T", [MLP, C], dt.bfloat16, kind="ExternalInput")
    b1_d = nc.dram_tensor("bias_fc1", [MLP], dt.float32, kind="ExternalInput")
    b2_d = nc.dram_tensor("bias_fc2", [C], dt.float32, kind="ExternalInput")
    rpb_d = nc.dram_tensor("rpb8", [KC, 128, 2, MCHUNK, PW], dt.float8e4,
                           kind="ExternalInput")
    id_d = nc.dram_tensor("ident8", [128, 128], dt.float8e4,
                          kind="ExternalInput")
    out_d = nc.dram_tensor("outT", [BPC, C, NPAD], dt.float32,
                           kind="ExternalOutput")

    x2_d = nc.dram_tensor("x2T_scratch", [BPC, C, NPAD], dt.float32)
    rec_d = nc.dram_tensor("rec_scratch", [BPC, NH, PW], dt.float32)
    rsd2_d = nc.dram_tensor("rstd2_scratch", [BPC, NPAD], dt.float32)

    with tile.TileContext(nc) as tc, ExitStack() as ctx:
        psA = ctx.enter_context(tc.tile_pool(name="psA", bufs=2, space="PSUM"))
        psPV = ctx.enter_context(tc.tile_pool(name="psPV", bufs=1, space="PSUM"))
        psG = ctx.enter_context(tc.tile_pool(name="psG", bufs=1, space="PSUM"))

        persist = ctx.enter_context(tc.tile_pool(name="persist", bufs=1))
        wqk_sb = persist.tile([128, KC, 2 * C], F8T, tag="wqk")
        nc.sync.dma_start(wqk_sb[:], wqk_d.ap().rearrange("(k p) c -> p k c", p=128))
        wv_sb = persist.tile([128, KC, C], F8T, tag="wv")
        nc.sync.dma_start(wv_sb[:], wv_d.ap().rearrange("(k p) c -> p k c", p=128))
        wp_sb = persist.tile([128, KC, C], F8T, tag="wp")
        nc.sync.dma_start(wp_sb[:], wp_d.ap().rearrange("(k p) c -> p k c", p=128))
        ones_sb = persist.tile([128, 1], BF16T, tag="ones")
        nc.vector.memset(ones_sb[:], 1.0)
        id_sb = persist.tile([128, 128], F8T, tag="ident")
        nc.sync.dma_start(id_sb[:], id_d.ap())
        eps_sb = persist.tile([128, 1], F32T, tag="eps")
        nc.vector.memset(eps_sb[:], EPS)
        bfc1_sb = persist.tile([128, MC], F32T, tag="bfc1")
        nc.sync.dma_start(bfc1_sb[:], b1_d.ap().rearrange("(m p) -> p m", p=128))
        bfc2_sb = persist.tile([128, KC], F32T, tag="bfc2")
        nc.sync.dma_start(bfc2_sb[:], b2_d.ap().rearrange("(m p) -> p m", p=128))

        xtp = ctx.enter_context(tc.tile_pool(name="xtp", bufs=2))
        qkp = ctx.enter_context(tc.tile_pool(name="qkp", bufs=2))
        kzp = ctx.enter_context(tc.tile_pool(name="kzp", bufs=2))
        v8p = ctx.enter_context(tc.tile_pool(name="v8p", bufs=2))
        ptp = ctx.enter_context(tc.tile_pool(name="ptp", bufs=2))
        rpbp = ctx.enter_context(tc.tile_pool(name="rpbp", bufs=2))
        osbp = ctx.enter_context(tc.tile_pool(name="osbp", bufs=2))
        onp = ctx.enter_context(tc.tile_pool(name="onp", bufs=2))
        xh2p = ctx.enter_context(tc.tile_pool(name="xh2p", bufs=2))
        mtp = ctx.enter_context(tc.tile_pool(name="mtp", bufs=1))
        w1tp = ctx.enter_context(tc.tile_pool(name="w1tp", bufs=2))
        w2tp = ctx.enter_context(tc.tile_pool(name="w2tp", bufs=2))
        ckp = ctx.enter_context(tc.tile_pool(name="ckp", bufs=2))
        ckm = ctx.enter_context(tc.tile_pool(name="ckm", bufs=2))
        rowp = ctx.enter_context(tc.tile_pool(name="rowp", bufs=2))
        rbp = ctx.enter_context(tc.tile_pool(name="rbp", bufs=2))
        smallp = ctx.enter_context(tc.tile_pool(name="smallp", bufs=2))
        denp = ctx.enter_context(tc.tile_pool(name="denp", bufs=1))

        # pool-slot zero-persistence counters (one per pool; slots alternate
        # and the zeroed regions are never overwritten by data)
        v8_cnt = [0]
        pt_cnt = [0]
        on_cnt = [0]

        # ---------------- qkv production for one item ----------------
        def qkv_steps(b):
            cell = {}

            def alloc_step():
                xt = xtp.tile([128, KC, NPAD], F8T, tag="xT")
                nc.sync.dma_start(
                    xt[:], xT_d.ap()[b].rearrange("(k p) t -> p k t", p=128))
                rrow = rowp.tile([128, NPAD], F32T, tag="rrow")
                src = rstd1_d.ap()[b]
                nc.sync.dma_start(rrow[:], bass.AP(
                    tensor=src.tensor, offset=src.offset,
                    ap=[[0, 128]] + list(src.ap)))
                rcol = smallp.tile([128, MCHUNK], F32T, tag="rcol")
                nc.sync.dma_start(
                    rcol[:], rstd1_d.ap()[b].rearrange("(c p) -> p c", p=128))
                qkT = qkp.tile([128, KC, PW], BF16T, tag="qkT")
                kz = kzp.tile([128, KC, PW], BF16T, tag="kz")
                v8 = v8p.tile([128, MCHUNK, VROW], F8T, tag="v8")
                if v8_cnt[0] < 2 or ZERO_ALL_SLABS:
                    v8_cnt[0] += 1
                    nc.vector.memset(v8[:], 0.0)
                    # ones columns (softmax denominator)
                    nc.vector.memset(
                        v8[:, :, 0:NH * 66].rearrange(
                            "p m (h e) -> p m h e", e=66)[:, :, :, 64:66], 1.0)
                cell.update(xT=xt, rrow=rrow, rcol=rcol, qkT=qkT, kz=kz, v8=v8)

            def qk_step(hp, which):
                ps = psG.tile([128, 1024], F32T, tag="ps")
                c0 = which * C + hp * 128
                for p in range(3):
                    for (lo, w) in SPL_N:
                        nc.tensor.matmul(
                            ps[:, lo:lo + w],
                            lhsT=wqk_sb[:, 2 * p:2 * p + 2, c0:c0 + 128],
                            rhs=cell['xT'][:, 2 * p:2 * p + 2, lo:lo + w],
                            start=(p == 0), stop=(p == 2), perf_mode=DR)
                dest = cell['qkT'] if which == 0 else cell['kz']
                nc.vector.tensor_tensor(
                    dest[:, hp, 0:N], ps[:, 0:N], cell['rrow'][:, 0:N], OP.mult)

            def v_step(mc):
                mw = 128 if mc < MCHUNK - 1 else N - 512
                ps = psG.tile([128, 1024], F32T, tag="ps")
                for p in range(3):
                    for (lo, w) in [(0, 512), (512, 256)]:
                        nc.tensor.matmul(
                            ps[0:mw, lo:lo + w],
                            lhsT=cell['xT'][:, 2 * p:2 * p + 2,
                                            mc * 128:mc * 128 + mw],
                            rhs=wv_sb[:, 2 * p:2 * p + 2, lo:lo + w],
                            start=(p == 0), stop=(p == 2), perf_mode=DR)
                nc.vector.tensor_scalar(
                    out=cell['v8'][0:mw, mc, 0:NH * 66].rearrange(
                        "p (h e) -> p h e", e=66)[:, :, 0:64],
                    in0=ps[0:mw, 0:C].rearrange("p (h e) -> p h e", h=NH),
                    scalar1=cell['rcol'][0:mw, mc:mc + 1], scalar2=None,
                    op0=OP.mult)

            steps = [alloc_step]
            for hp in range(KC):
                steps.append(lambda hp=hp: qk_step(hp, 0))
                steps.append(lambda hp=hp: qk_step(hp, 1))
            for mc in range(MCHUNK):
                steps.append(lambda mc=mc: v_step(mc))
            return steps, cell

        # ---------------- PV drain context ----------------
        class PvCtx:
            def __init__(self, pt, v8, h, osb, den12):
                self.pt, self.v8, self.h = pt, v8, h
                self.osb, self.den12 = osb, den12
                self.pv = None
                self.mms = [(p, lo, w) for p in range(3) for (lo, w) in SPL_N]
                self.pos = 0

            def drain(self, k):
                if self.pv is None:
                    self.pv = psPV.tile([128, 1024], F32T, tag="pv")
                h, j = self.h, self.h % 2
                w0 = h * 66
                mtail = N - 512
                end = min(self.pos + k, len(self.mms))
                for (p, lo, w) in self.mms[self.pos:end]:
                    if p < 2:
                        nc.tensor.matmul(
                            self.pv[:, lo:lo + w],
                            lhsT=self.v8[:, 2 * p:2 * p + 2, w0:w0 + 128],
                            rhs=self.pt[:, j, 2 * p:2 * p + 2, lo:lo + w],
                            start=(p == 0), stop=False, perf_mode=DR)
                    else:
                        nc.tensor.matmul(
                            self.pv[:, lo:lo + w],
                            lhsT=self.v8[0:mtail, 4, w0:w0 + 128],
                            rhs=self.pt[0:mtail, j, 4, lo:lo + w],
                            start=False, stop=True)
                self.pos = end
                if self.pos == len(self.mms):
                    hp = h // 2
                    dd = smallp.tile([1, PW], F32T, tag="dd")
                    nc.scalar.activation(dd[:, 0:N], self.pv[64:65, 0:N],
                                         AF.Identity)
                    nc.sync.dma_start(
                        self.den12[h:h + 1, 0:N], dd[:, 0:N])
                    nc.vector.tensor_copy(
                        self.osb[64 * j:64 * j + 64, hp, 0:N],
                        self.pv[0:64, 0:N])
                    self.pv = None
                    return True
                return False

            def finish(self):
                while self.pos < len(self.mms):
                    self.drain(6)

        # ---------------- attention phase for one item ----------------
        def attn_phase(b, cell, sec_mid, sec_tail):
            qkT, kz, v8 = cell['qkT'], cell['kz'], cell['v8']
            osb = osbp.tile([128, KC, PW], F8T, tag="osb")
            den12 = denp.tile([NH, PW], F32T, tag="den12")
            rts = {}

            def load_rpb(hp):
                rt = rpbp.tile([128, 2, MCHUNK, PW], F8T, tag="rpb")
                nc.sync.dma_start(rt[:], rpb_d.ap()[hp])
                rts[hp] = rt

            load_rpb(0)
            pend = []
            sec_i = [0]

            def run_sec(n):
                for _ in range(n):
                    if sec_i[0] < len(sec_mid):
                        sec_mid[sec_i[0]]()
                        sec_i[0] += 1

            for hp in range(KC):
                if hp + 1 < KC:
                    load_rpb(hp + 1)
                pt = ptp.tile([128, 2, MCHUNK, PW], F8T, tag="pt")
                for mc in range(MCHUNK):
                    mw = 128 if mc < MCHUNK - 1 else N - 512
                    m0 = mc * 128
                    sp2 = []
                    for j in (0, 1):  # row-tiled K=64 pair, concurrent
                        sps = psA.tile([128, 1024], F32T, tag="ps")
                        sp2.append(sps)
                        b0 = 64 * j
                        for (lo, w) in SPL_N:
                            nc.tensor.matmul(
                                sps[0:mw, lo:lo + w],
                                lhsT=kz[b0:b0 + 64, hp, m0:m0 + mw],
                                rhs=qkT[b0:b0 + 64, hp, lo:lo + w],
                                start=True, stop=False)
                    for j in (0, 1):  # rpb accumulate via fp8 identity
                        sps = sp2[j]
                        for (lo, w) in SPL_N:
                            nc.tensor.matmul(
                                sps[0:mw, lo:lo + w],
                                lhsT=id_sb[0:mw, 0:mw],
                                rhs=rts[hp][0:mw, j, mc, lo:lo + w],
                                start=False, stop=True)
                        nc.scalar.activation(
                            pt[0:mw, j, mc, 0:N], sps[0:mw, 0:N], AF.Exp)
                    if pend:
                        if pend[0].drain(2):
                            pend.pop(0)
                    run_sec(1)
                pend.append(PvCtx(pt, v8, 2 * hp, osb, den12))
                pend.append(PvCtx(pt, v8, 2 * hp + 1, osb, den12))
                while len(pend) > 2:
                    if pend[0].drain(6):
                        pend.pop(0)
            while pend:
                if pend[0].drain(6):
                    pend.pop(0)
                run_sec(1)
            rec12 = denp.tile([NH, PW], F32T, tag="rec12")
            nc.vector.reciprocal(rec12[:, 0:N], den12[:, 0:N])
            nc.sync.dma_start(rec_d.ap()[b][:, 0:N], rec12[:, 0:N])
            while sec_i[0] < len(sec_mid):
                sec_mid[sec_i[0]]()
                sec_i[0] += 1
            for s in sec_tail:
                s()
            return dict(osb=osb, b=b)

        # ---------------- finalize (runs inside next attn phase) -------
        def finalize_steps(fc):
            b, osb = fc['b'], fc['osb']
            onorm = onp.tile([128, KC, PW], F8T, tag="onorm")
            mid = []

            def mult_pair(hp):
                rb = rbp.tile([128, 2, PW], F32T, tag="rb")
                src = rec_d.ap()[b][2 * hp:2 * hp + 2, 0:N]
                nc.sync.dma_start(rb[:, :, 0:N], bass.AP(
                    tensor=src.tensor, offset=src.offset,
                    ap=[[0, 128]] + list(src.ap)))
                for j in (0, 1):
                    nc.vector.tensor_tensor(
                        onorm[64 * j:64 * j + 64, hp, 0:N],
                        osb[64 * j:64 * j + 64, hp, 0:N],
                        rb[64 * j:64 * j + 64, j, 0:N], OP.mult)
            for hp in range(KC):
                mid.append(lambda hp=hp: mult_pair(hp))

            stats = {}

            def proj_step(co):
                ps = psG.tile([128, 1024], F32T, tag="ps")
                for p in range(3):
                    for (lo, w) in SPL_N:
                        nc.tensor.matmul(
                            ps[:, lo:lo + w],
                            lhsT=wp_sb[:, 2 * p:2 * p + 2,
                                       co * 128:co * 128 + 128],
                            rhs=onorm[:, 2 * p:2 * p + 2, lo:lo + w],
                            start=(p == 0), stop=(p == 2), perf_mode=DR)
                xb = ckp.tile([128, PW], F32T, tag="xb")
                nc.sync.dma_start(
                    xb[:, 0:N], xbT_d.ap()[b][co * 128:(co + 1) * 128, 0:N])
                x2t = ckp.tile([128, PW], F32T, tag="x2t")
                nc.vector.tensor_tensor(
                    x2t[:, 0:N], ps[:, 0:N], xb[:, 0:N], OP.add)
                nc.sync.dma_start(
                    x2_d.ap()[b][co * 128:(co + 1) * 128, 0:N], x2t[:, 0:N])
                x2b = ckp.tile([128, PW], BF16T, tag="x2b")
                nc.vector.tensor_copy(x2b[:, 0:N], x2t[:, 0:N])
                sq = ckp.tile([128, PW], BF16T, tag="sq")
                nc.vector.tensor_tensor(
                    sq[:, 0:N], x2b[:, 0:N], x2b[:, 0:N], OP.mult)
                # per-co token sums (rows 0 and 32 of one psum tile);
                # accumulated on DVE so no PSUM tile persists across steps
                st = psG.tile([128, 1024], F32T, tag="ps", name="stat")
                for (lo, w) in SPL_N:
                    nc.tensor.matmul(
                        st[0:1, lo:lo + w], lhsT=ones_sb[:, 0:1],
                        rhs=x2b[:, lo:lo + w], start=True, stop=True)
                    nc.tensor.matmul(
                        st[32:33, lo:lo + w], lhsT=ones_sb[:, 0:1],
                        rhs=sq[:, lo:lo + w], start=True, stop=True,
                        tile_position=(0, 32))
                if co == 0:
                    nc.vector.tensor_copy(stats['sa'][:, 0:N], st[0:1, 0:N])
                    nc.vector.tensor_copy(stats['qa'][:, 0:N], st[32:33, 0:N])
                else:
                    nc.vector.tensor_tensor(
                        stats['sa'][:, 0:N], stats['sa'][:, 0:N],
                        st[0:1, 0:N], OP.add)
                    nc.vector.tensor_tensor(
                        stats['qa'][:, 0:N], stats['qa'][:, 0:N],
                        st[32:33, 0:N], OP.add)

            stats['sa'] = denp.tile([1, PW], F32T, tag="sa", name="sa_acc")
            stats['qa'] = denp.tile([1, PW], F32T, tag="qa", name="qa_acc")
            for co in range(KC):
                mid.append(lambda co=co: proj_step(co))

            def rstd2_step():
                ra = denp.tile([1, PW], F32T, tag="ra")
                rb2 = denp.tile([1, PW], F32T, tag="rb2")
                nc.vector.tensor_scalar(
                    out=ra[:, 0:N], in0=stats['sa'][:, 0:N],
                    scalar1=1.0 / C, scalar2=None, op0=OP.mult)
                nc.vector.tensor_tensor(                  # rb2 = mean^2
                    rb2[:, 0:N], ra[:, 0:N], ra[:, 0:N], OP.mult)
                nc.vector.scalar_tensor_tensor(           # ra = var
                    out=ra[:, 0:N], in0=stats['qa'][:, 0:N], scalar=1.0 / C,
                    in1=rb2[:, 0:N], op0=OP.mult, op1=OP.subtract)
                nc.scalar.activation(rb2[:, 0:N], ra[:, 0:N], AF.Ln,
                                     bias=eps_sb[0:1, 0:1])
                nc.scalar.activation(ra[:, 0:N], rb2[:, 0:N], AF.Exp,
                                     scale=-0.5)
                nc.sync.dma_start(
                    rsd2_d.ap()[b][0:N].rearrange("(o c) -> o c", o=1),
                    ra[:, 0:N])
            mid.append(rstd2_step)

            xh2 = xh2p.tile([128, KC, PW], F8T, tag="xh2")
            fc['xh2'] = xh2

            def xh2_step(co):
                if co == 0:
                    r2bc = rowp.tile([128, PW], F32T, tag="r2bc")
                    src = rsd2_d.ap()[b][0:N]
                    nc.sync.dma_start(r2bc[:, 0:N], bass.AP(
                        tensor=src.tensor, offset=src.offset,
                        ap=[[0, 128]] + list(src.ap)))
                    stats['r2bc'] = r2bc
                x2r = ckm.tile([128, PW], F32T, tag="x2r")
                nc.sync.dma_start(
                    x2r[:, 0:N],
                    x2_d.ap()[b][co * 128:(co + 1) * 128, 0:N])
                nc.vector.tensor_tensor(
                    xh2[:, co, 0:N], x2r[:, 0:N], stats['r2bc'][:, 0:N],
                    OP.mult)
            for co in range(KC):
                mid.append(lambda co=co: xh2_step(co))
            return mid, []

        # ---------------- MLP phase for one item ----------------
        def mlp_phase(b, fc):
            xh2 = fc['xh2']
            mt = mtp.tile([128, MC, PW], BF16T, tag="mt")
            w1ts = {}

            def load_w1(mc):
                t = w1tp.tile([128, KC, 128], F8T, tag="w1t")
                nc.sync.dma_start(
                    t[:], w1_d.ap().rearrange("(k p) c -> p k c", p=128)
                    [:, :, mc * 128:(mc + 1) * 128])
                w1ts[mc] = t

            load_w1(0)
            pspools = [psA, psA, psG]
            for mc in range(MC):
                if mc + 1 < MC:
                    load_w1(mc + 1)
                ps = pspools[mc % 3].tile([128, 1024], F32T, tag="ps")
                for p in range(3):
                    for (lo, w) in SPL_N:
                        nc.tensor.matmul(
                            ps[:, lo:lo + w],
                            lhsT=w1ts[mc][:, 2 * p:2 * p + 2, :],
                            rhs=xh2[:, 2 * p:2 * p + 2, lo:lo + w],
                            start=(p == 0), stop=(p == 2), perf_mode=DR)
                nc.scalar.activation(mt[:, mc, 0:N], ps[:, 0:N], GELU_AF,
                                     bias=bfc1_sb[:, mc:mc + 1])
                del w1ts[mc]

            w2ts = {}
            x2rs = {}

            def load_w2(co):
                t = w2tp.tile([128, MC, 128], BF16T, tag="w2t")
                nc.sync.dma_start(
                    t[:], w2_d.ap().rearrange("(k p) c -> p k c", p=128)
                    [:, :, co * 128:(co + 1) * 128])
                w2ts[co] = t
                xr = ckm.tile([128, PW], F32T, tag="x2r")
                nc.sync.dma_start(
                    xr[:, 0:N],
                    x2_d.ap()[b][co * 128:(co + 1) * 128, 0:N])
                x2rs[co] = xr

            load_w2(0)
            for co in range(KC):
                if co + 1 < KC:
                    load_w2(co + 1)
                ps = pspools[co % 3].tile([128, 1024], F32T, tag="ps")
                for k in range(MC):
                    for (lo, w) in SPL_N:
                        nc.tensor.matmul(
                            ps[:, lo:lo + w],
                            lhsT=w2ts[co][:, k, :],
                            rhs=mt[:, k, lo:lo + w],
                            start=(k == 0), stop=(k == MC - 1))
                ot = ckm.tile([128, PW], F32T, tag="ot")
                nc.vector.scalar_tensor_tensor(
                    out=ot[:, 0:N], in0=ps[:, 0:N],
                    scalar=bfc2_sb[:, co:co + 1], in1=x2rs[co][:, 0:N],
                    op0=OP.add, op1=OP.add)
                nc.sync.dma_start(
                    out_d.ap()[b][co * 128:(co + 1) * 128, 0:N], ot[:, 0:N])
                del w2ts[co], x2rs[co]

        # ---------------- orchestration ----------------
        def interleave(a, bl):
            out = []
            ia = ib = 0
            while ia < len(a) or ib < len(bl):
                if ib < len(bl):
                    out.append(bl[ib]); ib += 1
                if ia < len(a):
                    out.append(a[ia]); ia += 1
            return out

        q_steps, q_cell = qkv_steps(0)
        for s in q_steps:
            s()
        cells = {0: q_cell}
        fin_prev = None
        for b in range(BPC):
            sec_mid = []
            if b + 1 < BPC:
                qs, qc = qkv_steps(b + 1)
                cells[b + 1] = qc
                sec_mid = qs
            if fin_prev is not None:
                fmid, ftail = finalize_steps(fin_prev)
                sec_mid = interleave(fmid, sec_mid)
                sec_tail = ftail
            else:
                sec_tail = []
            fcell = attn_phase(b, cells[b], sec_mid, sec_tail)
            if fin_prev is not None:
                mlp_phase(fin_prev['b'], fin_prev)
            fin_prev = fcell
        fmid, ftail = finalize_steps(fin_prev)
        for s in fmid:
            s()
        for s in ftail:
            s()
        mlp_phase(BPC - 1, fin_prev)


def host_prep(inputs):
    """Fold LN gamma/means/scale into weights; channel-major layouts."""
    x = np.asarray(inputs['x'], F32)
    qkv_w = np.asarray(inputs['qkv_w'], F32)
    g1 = np.asarray(inputs['norm1_g'], F32)
    b1 = np.asarray(inputs['norm1_b'], F32)
    q_bias = np.asarray(inputs['q_bias'], F32)
    v_bias = np.asarray(inputs['v_bias'], F32)
    rpb_table = np.asarray(inputs['rpb_table'], F32)
    rel_index = np.asarray(inputs['rel_index'])
    proj_w = np.asarray(inputs['proj_w'], F32)
    proj_b = np.asarray(inputs['proj_b'], F32)
    g2 = np.asarray(inputs['norm2_g'], F32)
    b2 = np.asarray(inputs['norm2_b'], F32)
    fc1_w = np.asarray(inputs['fc1_w'], F32)
    fc1_b = np.asarray(inputs['fc1_b'], F32)
    fc2_w = np.asarray(inputs['fc2_w'], F32)
    fc2_b = np.asarray(inputs['fc2_b'], F32)

    assert np.allclose(b1, 0) and np.allclose(q_bias, 0) \
        and np.allclose(v_bias, 0) and np.allclose(b2, 0), \
        "zero-bias fast path only"

    Wq = qkv_w[0:C] * g1[None, :] * SCALE
    Wk = qkv_w[C:2 * C] * g1[None, :]
    Wv = qkv_w[2 * C:] * g1[None, :]
    W1 = fc1_w * g2[None, :]
    # rank-1 LN-mean fold
    Wq = Wq - Wq.mean(1, keepdims=True)
    Wk = Wk - Wk.mean(1, keepdims=True)
    Wv = Wv - Wv.mean(1, keepdims=True)
    W1 = W1 - W1.mean(1, keepdims=True)

    wqk8 = np.ascontiguousarray(np.concatenate([Wq, Wk], 0).T).astype(F8)
    wv8 = np.ascontiguousarray(Wv.T).astype(F8)
    wp8 = np.ascontiguousarray(proj_w.T).astype(F8)
    w18 = np.ascontiguousarray(W1.T).astype(F8)
    w2T = np.ascontiguousarray(fc2_w.T).astype(BF16)
    bias_fc1 = fc1_b.astype(F32)
    bias_fc2 = fc2_b.astype(F32)

    rpb = rpb_table[rel_index]                     # [N, N, NH]
    rpbT = rpb.transpose(2, 1, 0)                  # [H, m(key), n(query)]
    rpb8 = np.full((KC, 128, 2, MCHUNK, PW), -30.0, F32)
    for hp in range(KC):
        for j in range(2):
            for mc in range(MCHUNK):
                m0 = mc * 128
                mw = min(128, N - m0)
                rpb8[hp, 0:mw, j, mc, 0:N] = rpbT[2 * hp + j, m0:m0 + mw, :]
    rpb8 = rpb8.astype(F8)

    ident8 = np.eye(128, dtype=np.float32).astype(F8)

    xpad = np.zeros((B, NPAD, C), F32)
    xpad[:, :N, :] = x
    mean1 = xpad.mean(2)
    var1 = xpad.var(2)
    rstd1 = (1.0 / np.sqrt(var1 + EPS)).astype(F32)        # [B, NPAD]
    del mean1
    xT8 = np.ascontiguousarray(xpad.transpose(0, 2, 1)).astype(F8)
    xbT = np.ascontiguousarray(
        (xpad + proj_b[None, None, :]).transpose(0, 2, 1)).astype(F32)

    shared = dict(wqk8=wqk8, wv8=wv8, wp8=wp8, w18=w18, w2T=w2T,
                  bias_fc1=bias_fc1, bias_fc2=bias_fc2, rpb8=rpb8,
                  ident8=ident8)
    in_maps = []
    for core in range(N_CORES):
        sl = slice(core * BPC, (core + 1) * BPC)
        m = dict(shared)
        m['xT8'] = np.ascontiguousarray(xT8[sl])
        m['xbT'] = np.ascontiguousarray(xbT[sl])
        m['rstd1'] = np.ascontiguousarray(rstd1[sl])
        in_maps.append(m)
    return in_maps


def build_bass():
    nc = bacc.Bacc("TRN2", target_bir_lowering=False, debug=False,
                   num_devices=N_CORES)
    build_program(nc)
    nc.compile()
    return nc


def gather_output(results):
    out = np.zeros((B, N, C), F32)
    for core in range(N_CORES):
        o = results[core]["outT"]                   # [BPC, C, NPAD]
        out[core * BPC:(core + 1) * BPC] = o.transpose(0, 2, 1)[:, :N, :]
    return out


def kernel(**inputs):
    in_maps = host_prep(inputs)
    nc = build_bass()
    res = bass_utils.run_bass_kernel_spmd(nc, in_maps,
                                          core_ids=list(range(N_CORES)))
    return gather_output(res.results)
